# revision 9
# baseline (speedup 1.0000x reference)
"""Multi-head causal attention (B=2, T=2048, D=2048, H=16) on 8 trn2 NeuronCores.

Sharding: tensor-parallel over heads (2 heads/core). x^T is replicated, W_qkv
column-sliced and W_out row-sliced per core; each core computes a full-shape
partial of the output projection and the host sums the 8 partials (+ b_out).

v2: bf16 end-to-end. x/W_qkv/W_out and all intermediates (q,k,v,P,O) are bf16
(fp32 PSUM accumulation), halving HBM traffic and SBUF footprint; rel err
~6e-3 vs the 2e-2 gate. Scores are computed transposed [k, q] so softmax
rowsums need a ones-matmul; exp runs without max-subtraction (scores < ~25).
Causal structure: only lower-triangular tiles, with diagonal blocks shrunk to
N in {512,384,256,128} and the residual triangle zeroed by a DVE multiply
with a constant mask. The output projection is emitted per 512-token block
right after its O^T tiles are normalized, spreading stage-3 matmuls and y
DMAs across the attention phase instead of a serial tail. Engine placement:
ACT = exp + q/v bias evictions + half the y evictions; DVE = RoPE, k bias,
diag mask, normalization; Pool(gpsimd) = V^T->V eviction copies + half the y
evictions; all DMA on SP.
"""

import math
import os

import numpy as np

import concourse.bass as bass
import concourse.mybir as mybir
import concourse.tile as tile
from concourse import bacc
from concourse.bass_utils import run_bass_kernel_spmd

B, T, D_IN, D_MODEL, H = 2, 2048, 2048, 2048, 16
DH = 128
NCORES = 8
HPC = H // NCORES  # heads per core
BT = B * T
SCALE = 1.0 / math.sqrt(DH)

F32 = mybir.dt.float32
BF16 = mybir.dt.bfloat16
AF = mybir.ActivationFunctionType
ALU = mybir.AluOpType

TOKT = 512             # stage-1 token tile
NTT = T // TOKT        # token tiles per batch (4)
NDCH = D_IN // 128     # d_in contraction chunks (16)
NQ = T // 128          # 128-token chunks per batch (16)
NJ = T // 512          # q 512-tiles per batch (4)
NFT = D_MODEL // 512   # output feature tiles (4)


def build_nc(debug=False, reps=1):
    nc = bacc.Bacc("TRN2", target_bir_lowering=False, debug=False,
                   num_devices=NCORES)

    xT = nc.dram_tensor("xT", [D_IN, BT], BF16, kind="ExternalInput")
    wq = nc.dram_tensor("wq", [D_IN, HPC * DH], BF16, kind="ExternalInput")
    wk = nc.dram_tensor("wk", [D_IN, HPC * DH], BF16, kind="ExternalInput")
    wv = nc.dram_tensor("wv", [D_IN, HPC * DH], BF16, kind="ExternalInput")
    bq = nc.dram_tensor("bq", [HPC * DH], F32, kind="ExternalInput")
    bk = nc.dram_tensor("bk", [HPC * DH], F32, kind="ExternalInput")
    bv = nc.dram_tensor("bv", [HPC * DH], F32, kind="ExternalInput")
    wo = nc.dram_tensor("wo", [HPC * DH, D_MODEL], BF16, kind="ExternalInput")
    cosT = nc.dram_tensor("cosT", [DH, T], BF16, kind="ExternalInput")
    sinSW = nc.dram_tensor("sinSW", [DH, T], BF16, kind="ExternalInput")
    ident_d = nc.dram_tensor("ident", [128, 128], BF16, kind="ExternalInput")
    tri_d = nc.dram_tensor("tri", [128, 128], BF16, kind="ExternalInput")
    ones1_d = nc.dram_tensor("ones1", [1, 128], BF16, kind="ExternalInput")
    onescol_d = nc.dram_tensor("onescol", [128, 1], BF16, kind="ExternalInput")
    y = nc.dram_tensor("y", [BT, D_MODEL], BF16, kind="ExternalOutput")

    dbg = {}
    if debug:
        dbg["qT"] = nc.dram_tensor("dbg_qT", [HPC, B, DH, T], F32, kind="ExternalOutput")
        dbg["kT"] = nc.dram_tensor("dbg_kT", [HPC, B, DH, T], F32, kind="ExternalOutput")
        dbg["v"] = nc.dram_tensor("dbg_v", [B, T, HPC * DH], F32, kind="ExternalOutput")
        dbg["ot"] = nc.dram_tensor("dbg_ot", [B, HPC, DH, T], F32, kind="ExternalOutput")

    with tile.TileContext(nc) as tc:
        with (
            tc.tile_pool(name="persist", bufs=1) as pp,
            tc.tile_pool(name="weights", bufs=1) as wp,
            tc.tile_pool(name="qkv", bufs=1) as qp,
        ):
            # ---- per-core weights (persistent, outside the rep loop)
            wq_sb = wp.tile([128, NDCH, HPC * DH], BF16, name="wq_sb")
            wk_sb = wp.tile([128, NDCH, HPC * DH], BF16, name="wk_sb")
            wv_sb = wp.tile([128, NDCH, HPC * DH], BF16, name="wv_sb")
            wo_sb = wp.tile([128, HPC, D_MODEL], BF16, name="wo_sb")

            for hf in range(4):
                for t_, d_ in ((wq_sb, wq), (wk_sb, wk), (wv_sb, wv)):
                    nc.sync.dma_start(
                        t_[:, hf * (NDCH // 4):(hf + 1) * (NDCH // 4), :],
                        d_.ap()[hf * (D_IN // 4):(hf + 1) * (D_IN // 4), :]
                        .rearrange("(c p) f -> p c f", p=128))
            nc.sync.dma_start(wo_sb[:],
                              wo.ap().rearrange("(h p) f -> p h f", p=128))

            cosT_sb = pp.tile([DH, T], BF16, name="cosT_sb")
            sinSW_sb = pp.tile([DH, T], BF16, name="sinSW_sb")
            nc.sync.dma_start(cosT_sb[:], cosT.ap())
            nc.sync.dma_start(sinSW_sb[:], sinSW.ap())
            ones1 = pp.tile([1, 128], BF16, name="ones1")
            onescol = pp.tile([128, 1], BF16, name="onescol")
            ident = pp.tile([128, 128], BF16, name="ident")
            tri = pp.tile([128, 128], BF16, name="tri")
            nc.sync.dma_start(ones1[:], ones1_d.ap())
            nc.sync.dma_start(onescol[:], onescol_d.ap())
            nc.sync.dma_start(ident[:], ident_d.ap())
            nc.sync.dma_start(tri[:], tri_d.ap())
            bqt = pp.tile([128, HPC], F32, name="bqt")
            bkt = pp.tile([128, HPC], F32, name="bkt")
            bvt = pp.tile([128, HPC], F32, name="bvt")
            nc.sync.dma_start(bqt[:], bq.ap().rearrange("(h d) -> d h", d=DH))
            nc.sync.dma_start(bkt[:], bk.ap().rearrange("(h d) -> d h", d=DH))
            nc.sync.dma_start(bvt[:], bv.ap().rearrange("(h d) -> d h", d=DH))

            # ---- per-batch Q^T/K^T/V and O^T buffers (persistent)
            qT_sb = [qp.tile([DH, T], BF16, name=f"qT{h}") for h in range(HPC)]
            kT_sb = [qp.tile([DH, T], BF16, name=f"kT{h}") for h in range(HPC)]
            v_sb = qp.tile([128, NQ, HPC * DH], BF16, name="v_sb")
            ot_sb = [[qp.tile([DH, T], BF16, name=f"ot{b}_{h}") for h in range(HPC)]
                     for b in range(B)]

            import contextlib
            rep_ctx = (tc.For_i(0, reps, 1, hint_engines=(
                mybir.EngineType.PE, mybir.EngineType.Activation,
                mybir.EngineType.DVE, mybir.EngineType.Pool,
                mybir.EngineType.SP))
                if reps > 1 else contextlib.nullcontext())
            with rep_ctx:
                _emit_body(nc, tc, xT, wq_sb, wk_sb, wv_sb, bqt, bkt, bvt,
                           cosT_sb, sinSW_sb, qT_sb, kT_sb, v_sb, ot_sb,
                           wo_sb, y, ones1, onescol, ident, tri, dbg)
    nc.compile()
    return nc


def _emit_body(nc, tc, xT, wq_sb, wk_sb, wv_sb, bqt, bkt, bvt, cosT_sb,
               sinSW_sb, qT_sb, kT_sb, v_sb, ot_sb, wo_sb, y, ones1,
               onescol, ident, tri, dbg):
    with (
        tc.tile_pool(name="xs", bufs=3) as xs,
        tc.tile_pool(name="y_st", bufs=3) as ysp,
    ):
        for b in range(B):
            _stage1(nc, tc, b, xT, wq_sb, wk_sb, wv_sb, bqt, bkt, bvt,
                    cosT_sb, sinSW_sb, qT_sb, kT_sb, v_sb, ident, xs)
            if dbg:
                for h in range(HPC):
                    nc.sync.dma_start(dbg["qT"].ap()[h, b],
                                      qT_sb[h][:])
                    nc.sync.dma_start(dbg["kT"].ap()[h, b],
                                      kT_sb[h][:])
                nc.sync.dma_start(
                    dbg["v"].ap()[b].rearrange("(c p) f -> p c f", p=128),
                    v_sb[:])
            _stage23(nc, tc, b, qT_sb, kT_sb, v_sb, ones1, onescol, tri,
                     ot_sb, wo_sb, y, ysp, dbg)
        if dbg:
            for bb in range(B):
                for h in range(HPC):
                    nc.sync.dma_start(dbg["ot"].ap()[bb, h], ot_sb[bb][h][:])


def _stage1(nc, tc, b, xT, wq_sb, wk_sb, wv_sb, bqt, bkt, bvt,
            cosT_sb, sinSW_sb, qT_sb, kT_sb, v_sb, ident, xs):
    """QKV projection + RoPE for batch b: fills qT_sb/kT_sb/v_sb (bf16).

    Loop nest is d_in-chunk-outer so each x^T quarter-tile is touched once.
    q/k/v are computed transposed ([feat, tok], N=512); V is rotated back to
    natural [tok, feat] layout with PE transposes (stationary operand of P@V).
    RoPE: t1 = stg*cos, t2 = stg*sinSW (sign/swap folded into the table),
    out halves = t1_half + t2_otherhalf  -- all on DVE, no staging copies.
    """
    with (
        tc.tile_pool(name="st", bufs=3) as st,
        tc.tile_pool(name="vt", bufs=2) as vtp,
        tc.tile_pool(name="ps_qk", bufs=4, space="PSUM") as psqk,
        tc.tile_pool(name="ps_v", bufs=2, space="PSUM") as psv,
        tc.tile_pool(name="ps_tr", bufs=2, space="PSUM") as pstr,
    ):
        for tau in range(NTT):
            pos = tau * TOKT
            gtok = b * T + pos
            accs = [psqk.tile([128, TOKT], F32, name="qk_acc") for _ in range(4)]
            accvT = [psv.tile([128, TOKT], F32, name="vT_acc") for _ in range(2)]
            for quarter in range(4):
                xt = xs.tile([128, 4, TOKT], BF16, name="xt")
                nc.sync.dma_start(
                    xt[:],
                    xT.ap()[quarter * 512:(quarter + 1) * 512,
                            gtok:gtok + TOKT]
                    .rearrange("(c p) t -> p c t", p=128))
                for cl in range(4):
                    c = quarter * 4 + cl
                    for fi, (wsb, hh) in enumerate(
                            ((wq_sb, 0), (wq_sb, 1), (wk_sb, 0), (wk_sb, 1))):
                        nc.tensor.matmul(
                            accs[fi][:], wsb[:, c, hh * DH:(hh + 1) * DH],
                            xt[:, cl, :],
                            start=(c == 0), stop=(c == NDCH - 1))
                    for hh in range(HPC):
                        nc.tensor.matmul(
                            accvT[hh][:], wv_sb[:, c, hh * DH:(hh + 1) * DH],
                            xt[:, cl, :],
                            start=(c == 0), stop=(c == NDCH - 1))
            # q/k evictions with bias (split ACT/DVE), then RoPE on DVE
            for fi, (bias, dest, hh) in enumerate(
                    ((bqt, qT_sb, 0), (bqt, qT_sb, 1),
                     (bkt, kT_sb, 0), (bkt, kT_sb, 1))):
                stg = st.tile([128, TOKT], BF16, name="stg")
                if fi < 2:
                    nc.scalar.activation(stg[:], accs[fi][:], AF.Identity,
                                         bias=bias[:, hh:hh + 1], scale=1.0)
                else:
                    nc.vector.tensor_scalar_add(stg[:], accs[fi][:],
                                                bias[:, hh:hh + 1])
                rot = st.tile([128, TOKT], BF16, name="rot")
                nc.gpsimd.tensor_copy(rot[0:64, :], stg[64:128, :])
                nc.gpsimd.tensor_copy(rot[64:128, :], stg[0:64, :])
                nc.vector.tensor_tensor(
                    stg[:], stg[:], cosT_sb[:, pos:pos + TOKT], ALU.mult)
                nc.vector.tensor_tensor(
                    rot[:], rot[:], sinSW_sb[:, pos:pos + TOKT], ALU.mult)
                nc.vector.tensor_tensor(
                    dest[hh][:, pos:pos + TOKT], stg[:], rot[:], ALU.add)
            # V: evict V^T with bias (ACT), PE-transpose to natural layout,
            # land in v_sb via Pool copies
            for hh in range(HPC):
                vt = vtp.tile([128, TOKT], BF16, name="vt")
                nc.scalar.activation(vt[:], accvT[hh][:], AF.Identity,
                                     bias=bvt[:, hh:hh + 1], scale=1.0)
                for ts in range(4):
                    tr = pstr.tile([128, 128], BF16, name="tr")
                    nc.tensor.transpose(tr[:], vt[:, ts * 128:(ts + 1) * 128],
                                        ident[:])
                    nc.vector.tensor_copy(
                        v_sb[:, (pos // 128) + ts, hh * DH:(hh + 1) * DH],
                        tr[:])


def _stage23(nc, tc, b, qT_sb, kT_sb, v_sb, ones1, onescol, tri, ot_sb,
             wo_sb, y, ysp, dbg):
    """Causal attention for batch b + per-j output projection emission.

    Per (j, kk, h): S^T -> exp (ACT, bf16 out) -> diag-triangle zero (DVE
    mask multiply) -> P@V + ones-rowsum accumulation on PE. Diagonal blocks
    shrink N to 512-128*(kk%4). After each j: reciprocal + PE-broadcast of
    1/rowsum, O^T normalized on DVE; then the output projection for these
    512 tokens runs immediately (interleaving its matmuls and y DMAs with
    the next j's attention).
    """
    with (
        tc.tile_pool(name="spsB", bufs=2, space="PSUM") as spsB,
        tc.tile_pool(name="rps", bufs=2, space="PSUM") as rps,
        tc.tile_pool(name="ops", bufs=2, space="PSUM") as ops,
        tc.tile_pool(name="y_ps", bufs=2, space="PSUM") as yps,
        tc.tile_pool(name="scr", bufs=4) as scr,
        tc.tile_pool(name="pt_p", bufs=5) as ptp,
    ):
        for j in range(NJ):
            nkk = 4 * j + 4
            rp = [rps.tile([1, 512], F32, name="r_ps") for _ in range(HPC)]
            op = [ops.tile([128, 512], F32, name="o_ps") for _ in range(HPC)]
            for kk in range(nkk):
                diag = (kk // 4 == j)
                off = (kk % 4) * 128 if diag else 0
                n = 512 - off
                qlo = j * 512 + off
                for h in range(HPC):
                    qT, kT = qT_sb[h], kT_sb[h]
                    sp = spsB.tile([128, 512], F32, name="st_ps", tag="st_ps")
                    nc.tensor.matmul(sp[:, 0:n],
                                     kT[:, kk * 128:(kk + 1) * 128],
                                     qT[:, qlo:(j + 1) * 512],
                                     start=True, stop=True)
                    pt = ptp.tile([128, 512], BF16, name="pt")
                    nc.scalar.activation(pt[:, 0:n], sp[:, 0:n], AF.Exp,
                                         bias=0.0, scale=SCALE)
                    if diag:
                        # zero q < k inside the leading 128-col block
                        nc.gpsimd.tensor_tensor(pt[:, 0:128], pt[:, 0:128],
                                                tri[:], ALU.mult)
                    nc.tensor.matmul(op[h][:, off:512],
                                     v_sb[:, kk, h * DH:(h + 1) * DH],
                                     pt[:, 0:n], start=(kk == 0),
                                     stop=(kk == nkk - 1))
                    nc.tensor.matmul(rp[h][:, off:512], onescol[:],
                                     pt[:, 0:n], start=(kk == 0),
                                     stop=(kk == nkk - 1))
            # rowsum -> reciprocal -> broadcast across partitions -> evict
            for h in range(HPC):
                rrow_inv = scr.tile([1, 512], BF16, name="rrow_inv")
                with nc.allow_low_precision(reason="softmax denom to bf16"):
                    nc.vector.reciprocal(rrow_inv[:], rp[h][:])
                rb_ps = spsB.tile([128, 512], F32, name="st_ps", tag="st_ps")
                nc.tensor.matmul(rb_ps[:], ones1[:], rrow_inv[:],
                                 start=True, stop=True)
                rb = scr.tile([128, 512], F32, name="rb")
                nc.scalar.copy(rb[:], rb_ps[:])
                nc.vector.tensor_tensor(ot_sb[b][h][:, j * 512:(j + 1) * 512],
                                        op[h][:], rb[:], ALU.mult)
            # ---- output projection for tokens [j*512, (j+1)*512) ----------
            for tt in range(4):
                trow = j * 4 + tt
                yst = ysp.tile([128, D_MODEL], BF16, name="y_st")
                for ft in range(NFT):
                    ps = yps.tile([128, 512], F32, name="y_acc")
                    for h in range(HPC):
                        nc.tensor.matmul(
                            ps[:], ot_sb[b][h][:, trow * 128:(trow + 1) * 128],
                            wo_sb[:, h, ft * 512:(ft + 1) * 512],
                            start=(h == 0), stop=(h == HPC - 1))
                    if ft % 2 == 0:
                        nc.scalar.copy(yst[:, ft * 512:(ft + 1) * 512], ps[:])
                    else:
                        nc.vector.tensor_copy(yst[:, ft * 512:(ft + 1) * 512],
                                              ps[:])
                nc.sync.dma_start(
                    y.ap()[b * T + trow * 128:b * T + (trow + 1) * 128, :],
                    yst[:])


_CACHE = {}


def _get_nc():
    if "nc" not in _CACHE:
        _CACHE["nc"] = build_nc(debug=bool(int(os.environ.get("KERNEL_DEBUG", "0"))))
    return _CACHE["nc"]


def _host_prep(x, W_qkv, b_qkv, W_out, mask):
    from ml_dtypes import bfloat16
    xT = np.ascontiguousarray(x.reshape(BT, D_IN).T.astype(bfloat16))
    Wr = W_qkv.reshape(D_IN, H, 3, DH)
    br = b_qkv.reshape(H, 3, DH)
    # RoPE tables, transposed, sign-folded (rows 0:64 of sin negated) for the
    # half-swap rotate: q_rot = q*cos + swap_halves(q)*sinSW.
    inv_freq = (1.0 / (10000.0 ** (np.arange(0, DH, 2, dtype=np.float32) / DH))).astype(np.float32)
    tpos = np.arange(T, dtype=np.float32)
    freqs = tpos[:, None] * inv_freq[None, :]              # (T, 64)
    emb = np.concatenate([freqs, freqs], axis=-1)          # (T, 128)
    cosT = np.ascontiguousarray(np.cos(emb).astype(np.float32).T)
    sinSW = np.sin(emb).astype(np.float32).T               # (128, T)
    sinSW[0:64] = -sinSW[0:64]

    ident = np.eye(128, dtype=np.float32)
    tri_m = (np.arange(128)[None, :] >= np.arange(128)[:, None]).astype(np.float32)
    ones1 = np.ones((1, 128), dtype=np.float32)
    onescol = np.ones((128, 1), dtype=np.float32)

    in_maps = []
    for i in range(NCORES):
        hs = [HPC * i + k for k in range(HPC)]
        in_maps.append({
            "xT": xT,
            "wq": np.ascontiguousarray(Wr[:, hs, 0, :].reshape(D_IN, HPC * DH).astype(bfloat16)),
            "wk": np.ascontiguousarray(Wr[:, hs, 1, :].reshape(D_IN, HPC * DH).astype(bfloat16)),
            "wv": np.ascontiguousarray(Wr[:, hs, 2, :].reshape(D_IN, HPC * DH).astype(bfloat16)),
            "bq": np.ascontiguousarray(br[hs, 0, :].reshape(HPC * DH)),
            "bk": np.ascontiguousarray(br[hs, 1, :].reshape(HPC * DH)),
            "bv": np.ascontiguousarray(br[hs, 2, :].reshape(HPC * DH)),
            "wo": np.ascontiguousarray(W_out[hs[0] * DH:(hs[-1] + 1) * DH, :].astype(bfloat16)),
            "cosT": np.ascontiguousarray(cosT.astype(bfloat16)),
            "sinSW": np.ascontiguousarray(sinSW.astype(bfloat16)),
            "ident": ident.astype(bfloat16),
            "tri": tri_m.astype(bfloat16),
            "ones1": ones1.astype(bfloat16),
            "onescol": onescol.astype(bfloat16),
        })
    return in_maps


def kernel(x, W_qkv, b_qkv, W_out, b_out, mask):
    x = np.asarray(x, dtype=np.float32)
    in_maps = _host_prep(np.asarray(x), np.asarray(W_qkv), np.asarray(b_qkv),
                         np.asarray(W_out), np.asarray(mask))
    nc = _get_nc()
    res = run_bass_kernel_spmd(nc, in_maps, core_ids=list(range(NCORES)))
    out = np.asarray(res.results[0]["y"], dtype=np.float32)
    for i in range(1, NCORES):
        out += np.asarray(res.results[i]["y"], dtype=np.float32)
    out += np.asarray(b_out, dtype=np.float32)[None, :]
    return out.reshape(B, T, D_MODEL).astype(np.float32)
